# revision 1
# baseline (speedup 1.0000x reference)
"""ConditionalLM decode kernel for 8 Trainium2 NeuronCores.

Strategy:
  - Vocab-shard W_pred across 8 cores (4096 cols each, padded); shard stays
    SBUF-resident so the 65MB table is read from HBM exactly once.
  - GRU runs replicated (full batch) in transposed [feature, batch] layout so
    h_newT feeds the prediction matmul directly as lhsT.
  - Batch split into 2 independent decode streams (128 rows each), interleaved
    so each stream's argmax AllReduce hides under the other stream's compute.
  - Cross-core argmax: pack (sortable fp32 value, 0x7FFFFFFF - global_idx)
    into an int64 key, single AllReduce(max) per stream-step; ties resolve to
    the smallest vocab index, matching jnp.argmax.
  - All matmuls in fp32 (argmax margins down to 5e-8 forbid bf16/fp32r).
"""
import numpy as np

VOCAB = 32002
H = 512
COND = 1024
MAXLEN = 15
B = 256
NCORES = 8
NSHARD = 4096          # uniform per-core shard width (8*4096 = 32768 >= 32002)
NSTEPS = MAXLEN - 1    # 14 decode steps
P = 128
STREAMS = (0, 1)       # two batch halves


def _build(bcond_nz=False, bhn_nz=False):
    import concourse.bacc as bacc
    import concourse.mybir as mybir
    from concourse.tile import TileContext
    from concourse.bass import IndirectOffsetOnAxis

    f32 = mybir.dt.float32
    i32 = mybir.dt.int32
    u32 = mybir.dt.uint32
    i64 = mybir.dt.int64
    AF = mybir.ActivationFunctionType
    OP = mybir.AluOpType
    AxisX = mybir.AxisListType.X

    nc = bacc.Bacc("TRN2", target_bir_lowering=False, debug=True, num_devices=NCORES)

    # ---------------- I/O ----------------
    emb = nc.declare_dram_parameter("emb", [VOCAB, H], f32, isOutput=False)
    wpt = nc.declare_dram_parameter("wpt", [H, NSHARD], f32, isOutput=False)
    wiht = nc.declare_dram_parameter("wiht", [H, 3 * H], f32, isOutput=False)
    whht = nc.declare_dram_parameter("whht", [H, 3 * H], f32, isOutput=False)
    wct = nc.declare_dram_parameter("wct", [COND, H], f32, isOutput=False)
    imgT_d = nc.declare_dram_parameter("imgT", [COND, B], f32, isOutput=False)
    bcond = nc.declare_dram_parameter("bcond", [H], f32, isOutput=False)
    brz = nc.declare_dram_parameter("brz", [2 * H], f32, isOutput=False)
    bin_ = nc.declare_dram_parameter("bin", [H], f32, isOutput=False)
    bhn = nc.declare_dram_parameter("bhn", [H], f32, isOutput=False)
    tok0 = nc.declare_dram_parameter("tok0", [B], i32, isOutput=False)
    base_t = nc.declare_dram_parameter("base_t", [P, 1], i32, isOutput=False)
    ident_in = nc.declare_dram_parameter("ident_in", [P, P], f32, isOutput=False)
    if bcond_nz:
        bcond_row = nc.declare_dram_parameter("bcond_row", [1, H], f32, isOutput=False)
    if bhn_nz:
        bhn_row = nc.declare_dram_parameter("bhn_row", [1, H], f32, isOutput=False)
    ones_row = None
    preds = nc.declare_dram_parameter("preds", [B, MAXLEN], i32, isOutput=True)

    # internal DRAM for collectives (one pair per stream-step, static)
    # contribution: [128 rows, 2 words] = (val fp32, gidx int32) interleaved
    g_in = [[nc.dram_tensor(f"g_in_{t}_{s}", [P * 2], f32) for s in STREAMS]
            for t in range(NSTEPS)]
    g_out = [[nc.dram_tensor(f"g_out_{t}_{s}", [NCORES * P * 2], f32,
                             addr_space="Shared")
              for s in STREAMS] for t in range(NSTEPS)]

    KT = 4   # hidden k-tiles (512/128)
    KC = 8   # cond k-tiles (1024/128)
    NT = NSHARD // 512  # 8 pred n-tiles

    with TileContext(nc) as tc:
        with (
            tc.tile_pool(name="wts", bufs=1) as wts,       # resident weights
            tc.tile_pool(name="work", bufs=1) as work,     # per-stream state
            tc.tile_pool(name="sc", bufs=1) as sc,         # per-step scratch
            tc.tile_pool(name="ps", bufs=1, space="PSUM") as ps,
            tc.tile_pool(name="psr", bufs=3, space="PSUM") as psr,
        ):
            # ================= setup: load resident weights =================
            wpt_sb = [wts.tile([P, NSHARD], f32, tag=f"wpt{k}", name=f"wpt{k}") for k in range(KT)]
            wih_sb = [wts.tile([P, 3 * H], f32, tag=f"wih{k}", name=f"wih{k}") for k in range(KT)]
            whh_sb = [wts.tile([P, 3 * H], f32, tag=f"whh{k}", name=f"whh{k}") for k in range(KT)]

            base_sb = wts.tile([P, 1], i32, tag="base", name="base")
            nc.sync.dma_start(out=base_sb[:], in_=base_t[:])

            # per-partition bias columns: [128,1] slices
            brz_sb = [wts.tile([P, 1], f32, tag=f"brz{m}", name=f"brz{m}") for m in range(8)]
            for m in range(8):
                nc.sync.dma_start(out=brz_sb[m][:], in_=brz[m * P:(m + 1) * P][:, None])
            bin_sb = [wts.tile([P, 1], f32, tag=f"bin{m}", name=f"bin{m}") for m in range(KT)]
            for m in range(KT):
                nc.sync.dma_start(out=bin_sb[m][:], in_=bin_[m * P:(m + 1) * P][:, None])
            ones_sb = wts.tile([1, B], f32, tag="ones", name="ones")
            nc.vector.memset(ones_sb[:], 1.0)
            if bcond_nz:
                bcr_sb = wts.tile([1, H], f32, tag="bcr", name="bcr")
                nc.sync.dma_start(out=bcr_sb[:], in_=bcond_row[:])
            if bhn_nz:
                bhr_sb = wts.tile([1, H], f32, tag="bhr", name="bhr")
                nc.sync.dma_start(out=bhr_sb[:], in_=bhn_row[:])

            ident = wts.tile([P, P], f32, tag="ident", name="ident")
            nc.sync.dma_start(out=ident[:], in_=ident_in[:])

            # preds column 0 = seed tokens (DRAM->DRAM strided)
            with nc.allow_non_contiguous_dma(reason="column write, 256x4B"):
                nc.sync.dma_start(out=preds[:, 0][:, None], in_=tok0[:][:, None])

            # initial tokens per stream: SBUF [128,1] int32
            tok_sb = [work.tile([P, 1], i32, tag=f"tok{s}", name=f"tok{s}") for s in STREAMS]
            for s in STREAMS:
                nc.sync.dma_start(out=tok_sb[s][:], in_=tok0[s * P:(s + 1) * P][:, None])

            # hT per stream: packed [128(hidden k-tiles), 512] -> slice k*128
            hT = [work.tile([P, H], f32, tag=f"hT{s}", name=f"hT{s}") for s in STREAMS]

            # ================= h0 = W_cond @ imgT + b_cond =================
            with tc.tile_pool(name="setup", bufs=1) as setup:
                wct_sb = [setup.tile([P, H], f32, tag=f"wct{k}", name=f"wct{k}") for k in range(KC)]
                imgT_sb = [setup.tile([P, B], f32, tag=f"img{k}", name=f"img{k}") for k in range(KC)]
                for k in range(KC):
                    nc.sync.dma_start(out=wct_sb[k][:], in_=wct[k * P:(k + 1) * P, :])
                    nc.sync.dma_start(out=imgT_sb[k][:], in_=imgT_d[k * P:(k + 1) * P, :])

                for k in range(KT):
                    nc.sync.dma_start(out=wih_sb[k][:], in_=wiht[k * P:(k + 1) * P, :])
                    nc.sync.dma_start(out=whh_sb[k][:], in_=whht[k * P:(k + 1) * P, :])
                for k in range(KT):
                    nc.sync.dma_start(out=wpt_sb[k][:], in_=wpt[k * P:(k + 1) * P, :])
                for m in range(KT):
                    ps_h0 = psr.tile([P, 512], f32, tag="pred", name="pred")
                    for k in range(KC):
                        nc.tensor.matmul(
                            ps_h0[:, :B], lhsT=wct_sb[k][:, m * P:(m + 1) * P],
                            rhs=imgT_sb[k][:], start=(k == 0),
                            stop=(k == KC - 1 and not bcond_nz),
                        )
                    if bcond_nz:
                        nc.tensor.matmul(
                            ps_h0[:, :B], lhsT=bcr_sb[:, m * P:(m + 1) * P],
                            rhs=ones_sb[:, :B], start=False, stop=True)
                    for s in STREAMS:
                        nc.scalar.activation(hT[s][:, m * P:(m + 1) * P],
                                             ps_h0[:, s * P:(s + 1) * P], AF.Copy)

            # ================= decode steps =================
            logit_buf = [work.tile([P, NSHARD], f32, tag=f"logits{s}", name=f"logits{s}")
                         for s in STREAMS]
            for t in range(NSTEPS):
                for s in STREAMS:
                    # ---- gather x = emb[tok] : [128, 512] ----
                    x_sb = sc.tile([P, H], f32, tag=f"x{s}", name=f"x{s}")
                    nc.gpsimd.indirect_dma_start(
                        out=x_sb[:], out_offset=None, in_=emb[:],
                        in_offset=IndirectOffsetOnAxis(ap=tok_sb[s][:, :1], axis=0),
                    )
                    # ---- xT via PE transpose: 4 tiles [128,128] packed ----
                    ps_tp = ps.tile([P, 512], f32, tag="tp", name="tp")
                    xT = sc.tile([P, H], f32, tag=f"xT{s}", name=f"xT{s}")
                    for j in range(KT):
                        nc.tensor.transpose(ps_tp[:, j * P:(j + 1) * P],
                                            x_sb[:, j * P:(j + 1) * P], ident[:])
                    nc.vector.tensor_copy(xT[:], ps_tp[:])

                    # ---- GRU matmuls (transposed layout, N=128 batch) ----
                    # rz gates: psum[m] = sum_k wih[k][:,m]@xT[k] + whh[k][:,m]@hT[k]
                    ps_rz = [ps.tile([P, 512], f32, tag=f"rz{h}", name=f"rz{h}") for h in range(2)]
                    for m in range(8):  # m-tiles 0..7 cover r and z blocks
                        out_sl = ps_rz[m // 4][:, (m % 4) * P:(m % 4 + 1) * P]
                        for k in range(KT):
                            nc.tensor.matmul(
                                out_sl, lhsT=wih_sb[k][:, m * P:(m + 1) * P],
                                rhs=xT[:, k * P:(k + 1) * P],
                                start=(k == 0), stop=False,
                            )
                        for k in range(KT):
                            nc.tensor.matmul(
                                out_sl, lhsT=whh_sb[k][:, m * P:(m + 1) * P],
                                rhs=hT[s][:, k * P:(k + 1) * P],
                                start=False, stop=(k == KT - 1),
                            )
                    # n gates: separate i_n and h_n
                    ps_in = ps.tile([P, 512], f32, tag="in", name="in")
                    ps_hn = ps.tile([P, 512], f32, tag="hn", name="hn")
                    for m in range(KT):
                        g = 8 + m
                        for k in range(KT):
                            nc.tensor.matmul(
                                ps_in[:, m * P:(m + 1) * P],
                                lhsT=wih_sb[k][:, g * P:(g + 1) * P],
                                rhs=xT[:, k * P:(k + 1) * P],
                                start=(k == 0), stop=(k == KT - 1),
                            )
                        for k in range(KT):
                            nc.tensor.matmul(
                                ps_hn[:, m * P:(m + 1) * P],
                                lhsT=whh_sb[k][:, g * P:(g + 1) * P],
                                rhs=hT[s][:, k * P:(k + 1) * P],
                                start=(k == 0),
                                stop=(k == KT - 1 and not bhn_nz),
                            )
                        if bhn_nz:
                            nc.tensor.matmul(
                                ps_hn[:, m * P:(m + 1) * P],
                                lhsT=bhr_sb[:, m * P:(m + 1) * P],
                                rhs=ones_sb[:, :P], start=False, stop=True)

                    # ---- gates elementwise ----
                    r_sb = sc.tile([P, 512], f32, tag=f"r{s}", name=f"r{s}")
                    z_sb = sc.tile([P, 512], f32, tag=f"z{s}", name=f"z{s}")
                    hn_sb = sc.tile([P, 512], f32, tag=f"hn{s}", name=f"hn{s}")
                    for m in range(KT):
                        nc.scalar.activation(
                            r_sb[:, m * P:(m + 1) * P],
                            ps_rz[m // 4][:, (m % 4) * P:(m % 4 + 1) * P],
                            AF.Sigmoid, bias=brz_sb[m][:])
                    for m in range(KT):
                        mm = 4 + m
                        nc.scalar.activation(
                            z_sb[:, m * P:(m + 1) * P],
                            ps_rz[mm // 4][:, (mm % 4) * P:(mm % 4 + 1) * P],
                            AF.Sigmoid, bias=brz_sb[mm][:])
                    nc.scalar.activation(hn_sb[:], ps_hn[:], AF.Copy)
                    # t2 = r*hn + i_n ; n = tanh(t2 + b_in)
                    t2_sb = sc.tile([P, 512], f32, tag=f"t2{s}", name=f"t2{s}")
                    nc.vector.tensor_mul(t2_sb[:], r_sb[:], hn_sb[:])
                    nc.vector.tensor_add(t2_sb[:], t2_sb[:], ps_in[:])
                    n_sb = sc.tile([P, 512], f32, tag=f"n{s}", name=f"n{s}")
                    for m in range(KT):
                        nc.scalar.activation(n_sb[:, m * P:(m + 1) * P],
                                             t2_sb[:, m * P:(m + 1) * P],
                                             AF.Tanh, bias=bin_sb[m][:])
                    # h' = n + z*(h - n)
                    d_sb = sc.tile([P, 512], f32, tag=f"d{s}", name=f"d{s}")
                    nc.gpsimd.tensor_sub(d_sb[:], hT[s][:], n_sb[:])
                    nc.gpsimd.tensor_mul(d_sb[:], d_sb[:], z_sb[:])
                    nc.gpsimd.tensor_add(d_sb[:], d_sb[:], n_sb[:])
                    nc.vector.tensor_copy(hT[s][:], d_sb[:])

                    # ---- prediction matmul: logits[128, 4096] ----
                    for n in range(NT):
                        ps_pred = psr.tile([P, 512], f32, tag="pred", name="pred")
                        for k in range(KT):
                            nc.tensor.matmul(
                                ps_pred[:], lhsT=hT[s][:, k * P:(k + 1) * P],
                                rhs=wpt_sb[k][:, n * 512:(n + 1) * 512],
                                start=(k == 0), stop=(k == KT - 1),
                            )
                        nc.scalar.activation(logit_buf[s][:, n * 512:(n + 1) * 512],
                                             ps_pred[:], AF.Copy)

                    # ---- local argmax ----
                    m8 = sc.tile([P, 8], f32, tag=f"m8{s}", name=f"m8{s}")
                    mi = sc.tile([P, 8], u32, tag=f"mi{s}", name=f"mi{s}")
                    nc.vector.max(out=m8[:], in_=logit_buf[s][:])
                    nc.vector.max_index(out=mi[:], in_max=m8[:],
                                        in_values=logit_buf[s][:])

                    # ---- contribution (val, gidx) pairs; AllGather ----
                    key_sb = sc.tile([P, 2], f32, tag=f"key{s}", name=f"key{s}")
                    nc.vector.tensor_copy(key_sb[:, 0:1], m8[:, 0:1])
                    nc.vector.tensor_add(key_sb[:, 1:2].bitcast(i32),
                                         mi[:, 0:1].bitcast(i32), base_sb[:])
                    nc.sync.dma_start(
                        out=g_in[t][s][:].rearrange("(p w) -> p w", w=2),
                        in_=key_sb[:])
                    nc.gpsimd.collective_compute(
                        "AllGather", OP.bypass,
                        replica_groups=[list(range(NCORES))],
                        ins=[g_in[t][s][:]], outs=[g_out[t][s][:]],
                    )
                    # ---- local combine over 8 cores ----
                    # gathered word (c, p, w) at offset c*256 + p*2 + w
                    gv = g_out[t][s][:].rearrange("(c p w) -> p c w", c=NCORES, w=2)
                    vals8 = sc.tile([P, NCORES], f32, tag=f"v8{s}", name=f"v8{s}")
                    idx8 = sc.tile([P, NCORES], i32, tag=f"i8{s}", name=f"i8{s}")
                    nc.sync.dma_start(out=vals8[:], in_=gv[:, :, 0])
                    nc.sync.dma_start(out=idx8[:], in_=gv[:, :, 1].bitcast(i32))
                    gmax = sc.tile([P, 1], f32, tag=f"gm{s}", name=f"gm{s}")
                    nc.vector.tensor_reduce(gmax[:], vals8[:], AxisX, OP.max)
                    mask = sc.tile([P, NCORES], u32, tag=f"mk{s}", name=f"mk{s}")
                    nc.vector.tensor_tensor(mask[:], vals8[:],
                                            gmax[:].to_broadcast([P, NCORES]),
                                            OP.is_ge)
                    cand = sc.tile([P, NCORES], i32, tag=f"cd{s}", name=f"cd{s}")
                    nc.vector.memset(cand[:], 0x7FFFFFFF)
                    nc.vector.copy_predicated(cand[:], mask[:], idx8[:])
                    tok_new = work.tile([P, 1], i32, tag=f"tok{s}", name=f"tok{s}")
                    nc.vector.tensor_reduce(tok_new[:], cand[:], AxisX, OP.min)
                    tok_sb[s] = tok_new
                    # write preds[:, t+1] for this stream's rows
                    with nc.allow_non_contiguous_dma(reason="column write, 128x4B"):
                        nc.sync.dma_start(
                            out=preds[s * P:(s + 1) * P, t + 1][:, None],
                            in_=tok_new[:])

    return nc


def _prep_inputs(caption, img, embedding, W_cond, b_cond, w_ih, w_hh, b_ih,
                 b_hh, W_pred, b_pred):
    caption = np.asarray(caption).astype(np.int32)
    img = np.ascontiguousarray(np.asarray(img, dtype=np.float32))
    embedding = np.ascontiguousarray(np.asarray(embedding, dtype=np.float32))
    W_pred = np.asarray(W_pred, dtype=np.float32)
    b_ih = np.asarray(b_ih, np.float32)
    b_hh = np.asarray(b_hh, np.float32)
    common = dict(
        emb=embedding,
        wiht=np.ascontiguousarray(np.asarray(w_ih, np.float32).T),
        whht=np.ascontiguousarray(np.asarray(w_hh, np.float32).T),
        wct=np.ascontiguousarray(np.asarray(W_cond, np.float32).T),
        imgT=np.ascontiguousarray(img.T),
        bcond=np.asarray(b_cond, np.float32),
        brz=np.ascontiguousarray(b_ih[:2 * H] + b_hh[:2 * H]),
        bin=np.ascontiguousarray(b_ih[2 * H:]),
        bhn=np.ascontiguousarray(b_hh[2 * H:]),
        tok0=np.ascontiguousarray(caption[:, 0]),
        ident_in=np.eye(P, dtype=np.float32),
        bcond_row=np.asarray(b_cond, np.float32).reshape(1, H),
        bhn_row=np.ascontiguousarray(b_hh[2 * H:]).reshape(1, H),
    )
    in_maps = []
    for c in range(NCORES):
        base = c * NSHARD
        hi = min(base + NSHARD, VOCAB)
        n_real = max(0, hi - base)
        wpt_c = np.empty((H, NSHARD), np.float32)
        wpt_c[:, :n_real] = W_pred[base:hi].T
        if n_real < NSHARD:
            # pad columns duplicate column 0 of this shard; they tie with the
            # real entry and lose on index, so a pad can never win the argmax
            wpt_c[:, n_real:] = wpt_c[:, 0:1]
        m = dict(common)
        m["wpt"] = np.ascontiguousarray(wpt_c)
        m["base_t"] = np.full((P, 1), base, np.int32)
        in_maps.append(m)
    return in_maps


_CACHED = {}


def kernel(**inputs) -> np.ndarray:
    from concourse.bass_utils import run_bass_kernel_spmd

    in_maps = _prep_inputs(**inputs)
    bcond_nz = bool(np.any(np.asarray(inputs["b_cond"])))
    bhn_nz = bool(np.any(np.asarray(inputs["b_hh"])[2 * H:]))
    key = (bcond_nz, bhn_nz)
    if key not in _CACHED:
        nc = _build(*key)
        nc.finalize()
        _CACHED[key] = nc
    res = run_bass_kernel_spmd(_CACHED[key], in_maps, list(range(NCORES)))
    return np.ascontiguousarray(res.results[0]["preds"].astype(np.int32))


if __name__ == "__main__":
    d = np.load("inputs.npz")
    inputs = {k: d[k] for k in d.files}
    out = kernel(**inputs)
    exp = np.load("expected.npy")
    print("match:", np.array_equal(out, exp),
          " mismatches:", int((out != exp).sum()), "/", out.size)



# revision 5
# speedup vs baseline: 1.4603x; 1.4603x over previous
"""ConditionalLM decode kernel for 8 Trainium2 NeuronCores.

Strategy (v2):
  - Vocab-shard W_pred across 8 cores (4096 cols each, zero-padded); the
    prediction matmul runs in float32r (1 cyc/row vs 4 for fp32), and the
    top-2 local candidates are re-scored with an exact fp32 dot product on
    the vector engine so fp32r rounding (~3.5e-7 logit noise) cannot flip
    the argmax (measured top-2 margins go down to 5e-8).
  - GRU is sharded 8 ways by gate feature columns (64 h-features per core,
    permuted weight columns r_c|z_c|n_c uploaded per core), computed
    row-major in exact fp32, then h' slices are exchanged with a 32KB
    AllGather per stream-step.  h_old for the elementwise combine is
    selected with a one-hot matrix appended to the whh weight upload (SPMD
    programs cannot use per-core addresses).
  - Batch split into 2 streams of 128 rows, interleaved so each stream's
    collectives hide under the other stream's compute.
  - Cross-core argmax: (exact val, global idx) pairs, AllGather + local
    combine; ties resolve to the smallest vocab index, matching jnp.argmax.
"""
import numpy as np

VOCAB = 32002
H = 512
COND = 1024
MAXLEN = 15
B = 256
NCORES = 8
NSHARD = 4096          # per-core vocab shard (8*4096 = 32768 >= 32002)
NSTEPS = MAXLEN - 1    # 14 decode steps
P = 128
F = 64                 # h-features per core (512/8)
STREAMS = (0, 1)
KT = 4                 # hidden k-tiles (512/128)
KC = 8                 # cond k-tiles (1024/128)
NT = NSHARD // 512     # 8 pred n-tiles


def _build(bcond_nz=False, bgate_nz=False, bpred_nz=False):
    import concourse.bacc as bacc
    import concourse.mybir as mybir
    from concourse.tile import TileContext
    from concourse.bass import IndirectOffsetOnAxis

    f32 = mybir.dt.float32
    f32r = mybir.dt.float32r
    i32 = mybir.dt.int32
    u32 = mybir.dt.uint32
    AF = mybir.ActivationFunctionType
    OP = mybir.AluOpType
    AxisX = mybir.AxisListType.X

    nc = bacc.Bacc("TRN2", target_bir_lowering=False, debug=True, num_devices=NCORES)

    # ---------------- I/O ----------------
    emb = nc.declare_dram_parameter("emb", [VOCAB, H], f32, isOutput=False)
    wpt = nc.declare_dram_parameter("wpt", [H, NSHARD], f32, isOutput=False)
    wrows = nc.declare_dram_parameter("wrows", [NSHARD, H + 1], f32, isOutput=False)
    wih_r = nc.declare_dram_parameter("wih_r", [H, 3 * F], f32, isOutput=False)
    whh_r = nc.declare_dram_parameter("whh_r", [H, 3 * F + F], f32, isOutput=False)
    wct = nc.declare_dram_parameter("wct", [COND, H], f32, isOutput=False)
    imgT_d = nc.declare_dram_parameter("imgT", [COND, B], f32, isOutput=False)
    tok0 = nc.declare_dram_parameter("tok0", [B], i32, isOutput=False)
    base_t = nc.declare_dram_parameter("base_t", [P, 1], i32, isOutput=False)
    ident_in = nc.declare_dram_parameter("ident_in", [P, P], f32, isOutput=False)
    if bcond_nz:
        bcond_row = nc.declare_dram_parameter("bcond_row", [1, H], f32, isOutput=False)
    if bgate_nz:
        # permuted per-core gate biases: [rz (128) | i_n (64) | h_n (64)]
        bgate_row = nc.declare_dram_parameter("bgate_row", [1, 4 * F], f32, isOutput=False)
    if bpred_nz:
        bpred_row = nc.declare_dram_parameter("bpred_row", [1, NSHARD], f32, isOutput=False)
    preds = nc.declare_dram_parameter("preds", [B, MAXLEN], i32, isOutput=True)
    DEBUG = True
    if DEBUG:
        dbg_h = nc.declare_dram_parameter("dbg_h", [P, H + 1], f32, isOutput=True)
        dbg_m8 = nc.declare_dram_parameter("dbg_m8", [P, 8], f32, isOutput=True)
        dbg_mi = nc.declare_dram_parameter("dbg_mi", [P, 8], i32, isOutput=True)
        dbg_ev = nc.declare_dram_parameter("dbg_ev", [P, 2], f32, isOutput=True)
        dbg_key = nc.declare_dram_parameter("dbg_key", [P, 2], f32, isOutput=True)
        dbg_v8 = nc.declare_dram_parameter("dbg_v8", [P, NCORES], f32, isOutput=True)
        dbg_lg = nc.declare_dram_parameter("dbg_lg", [P, NSHARD], f32, isOutput=True)
        dbg_h0 = nc.declare_dram_parameter("dbg_h0", [P, H], f32, isOutput=True)
        dbg_x = nc.declare_dram_parameter("dbg_x", [P, H], f32, isOutput=True)
        dbg_xT = nc.declare_dram_parameter("dbg_xT", [P, H], f32, isOutput=True)
        dbg_hT = nc.declare_dram_parameter("dbg_hT", [P, H], f32, isOutput=True)
        dbg_r = nc.declare_dram_parameter("dbg_r", [P, F], f32, isOutput=True)
        dbg_z = nc.declare_dram_parameter("dbg_z", [P, F], f32, isOutput=True)
        dbg_n = nc.declare_dram_parameter("dbg_n", [P, F], f32, isOutput=True)
        dbg_ho = nc.declare_dram_parameter("dbg_ho", [P, F], f32, isOutput=True)

    # internal DRAM for collectives (one pair per stream-step, static)
    k_in = [[nc.dram_tensor(f"k_in_{t}_{s}", [P * 2], f32) for s in STREAMS]
            for t in range(NSTEPS)]
    k_out = [[nc.dram_tensor(f"k_out_{t}_{s}", [NCORES * P * 2], f32,
                             addr_space="Shared") for s in STREAMS]
             for t in range(NSTEPS)]
    h_in = [[nc.dram_tensor(f"h_in_{t}_{s}", [P * F], f32) for s in STREAMS]
            for t in range(NSTEPS)]
    h_out = [[nc.dram_tensor(f"h_out_{t}_{s}", [NCORES * P * F], f32,
                             addr_space="Shared") for s in STREAMS]
             for t in range(NSTEPS)]

    with TileContext(nc) as tc:
        with (
            tc.tile_pool(name="wts", bufs=1) as wts,
            tc.tile_pool(name="work", bufs=1) as work,
            tc.tile_pool(name="sc", bufs=1) as sc,
            tc.tile_pool(name="psg", bufs=1, space="PSUM") as psg,
            tc.tile_pool(name="pst", bufs=1, space="PSUM") as pst,
            tc.tile_pool(name="psr", bufs=4, space="PSUM") as psr,
        ):
            # ================= resident weights =================
            base_sb = wts.tile([P, 1], i32, tag="base", name="base")
            nc.sync.dma_start(out=base_sb[:], in_=base_t[:])
            ident = wts.tile([P, P], f32, tag="ident", name="ident")
            nc.sync.dma_start(out=ident[:], in_=ident_in[:])

            # GRU weights, k-tiles packed along free dim
            wih_sb = wts.tile([P, KT * 3 * F], f32, tag="wih", name="wih")
            whh_sb = wts.tile([P, KT * 4 * F], f32, tag="whh", name="whh")
            for k in range(KT):
                nc.sync.dma_start(out=wih_sb[:, k * 3 * F:(k + 1) * 3 * F],
                                  in_=wih_r[k * P:(k + 1) * P, :])
                nc.sync.dma_start(out=whh_sb[:, k * 4 * F:(k + 1) * 4 * F],
                                  in_=whh_r[k * P:(k + 1) * P, :])

            # prediction weights: stage fp32, cast to f32r
            wpt_r = [wts.tile([P, NSHARD], f32r, tag=f"wptr{k}", name=f"wptr{k}")
                     for k in range(KT)]
            with tc.tile_pool(name="stage", bufs=2) as stage:
                for k in range(KT):
                    st = stage.tile([P, NSHARD], f32, tag="st", name="st")
                    nc.sync.dma_start(out=st[:], in_=wpt[k * P:(k + 1) * P, :])
                    nc.vector.tensor_copy(wpt_r[k][:], st[:])

            if bcond_nz or bgate_nz or bpred_nz:
                ones_col = wts.tile([1, P], f32, tag="ones", name="ones")
                nc.vector.memset(ones_col[:], 1.0)
            if bcond_nz:
                bcr_sb = wts.tile([1, H], f32, tag="bcr", name="bcr")
                nc.sync.dma_start(out=bcr_sb[:], in_=bcond_row[:])
            if bgate_nz:
                bgr_sb = wts.tile([1, 4 * F], f32, tag="bgr", name="bgr")
                nc.sync.dma_start(out=bgr_sb[:], in_=bgate_row[:])
            if bpred_nz:
                ones_col_r = wts.tile([1, P], f32r, tag="onesr", name="onesr")
                nc.vector.memset(ones_col_r[:], 1.0)
                bpr_sb = wts.tile([1, NSHARD], f32r, tag="bpr", name="bpr")
                st2 = wts.tile([1, NSHARD], f32, tag="bprs", name="bprs")
                nc.sync.dma_start(out=st2[:], in_=bpred_row[:])
                nc.vector.tensor_copy(bpr_sb[:], st2[:])

            # preds column 0 = seed tokens
            with nc.allow_non_contiguous_dma(reason="column write, 256x4B"):
                nc.sync.dma_start(out=preds[:, 0][:, None], in_=tok0[:][:, None])

            tok_sb = [work.tile([P, 1], i32, tag=f"tok{s}", name=f"tok{s}")
                      for s in STREAMS]
            for s in STREAMS:
                nc.sync.dma_start(out=tok_sb[s][:], in_=tok0[s * P:(s + 1) * P][:, None])

            # persistent state: h row-major (col H holds 1.0 for the bias-
            # augmented rescore dot), hT fp32 and f32r (k-tiles along free)
            h_row = [work.tile([P, H + 1], f32, tag=f"hrow{s}", name=f"hrow{s}")
                     for s in STREAMS]
            hT = [work.tile([P, H], f32, tag=f"hT{s}", name=f"hT{s}")
                  for s in STREAMS]
            hTr = [work.tile([P, H], f32r, tag=f"hTr{s}", name=f"hTr{s}")
                   for s in STREAMS]
            xT = [work.tile([P, H], f32, tag=f"xT{s}", name=f"xT{s}")
                  for s in STREAMS]
            for s in STREAMS:
                nc.vector.memset(h_row[s][:, H:H + 1], 1.0)

            def transpose_to_hT(src_row, s):
                """4 PE transposes of src_row [128,512] -> hT/hTr [128k, batch]."""
                ps_tp = pst.tile([P, H], f32, tag=f"tp{s}", name=f"tp{s}")
                for j in range(KT):
                    nc.tensor.transpose(ps_tp[:, j * P:(j + 1) * P],
                                        src_row[:, j * P:(j + 1) * P], ident[:])
                nc.scalar.activation(hT[s][:], ps_tp[:], AF.Copy)
                nc.vector.tensor_copy(hTr[s][:], ps_tp[:])

            # ================= h0 = img @ W_cond.T (row-major) =================
            with tc.tile_pool(name="setup", bufs=1) as setup:
                wct_sb = [setup.tile([P, H], f32, tag=f"wct{k}", name=f"wct{k}")
                          for k in range(KC)]
                imgT_sb = [setup.tile([P, B], f32, tag=f"img{k}", name=f"img{k}")
                           for k in range(KC)]
                for k in range(KC):
                    nc.sync.dma_start(out=wct_sb[k][:], in_=wct[k * P:(k + 1) * P, :])
                    nc.sync.dma_start(out=imgT_sb[k][:], in_=imgT_d[k * P:(k + 1) * P, :])
                for s in STREAMS:
                    ps_h0 = psr.tile([P, H], f32, tag="pred", name="ps_h0")
                    for k in range(KC):
                        nc.tensor.matmul(
                            ps_h0[:], lhsT=imgT_sb[k][:, s * P:(s + 1) * P],
                            rhs=wct_sb[k][:], start=(k == 0),
                            stop=(k == KC - 1 and not bcond_nz))
                    if bcond_nz:
                        nc.tensor.matmul(ps_h0[:], lhsT=ones_col[:],
                                         rhs=bcr_sb[:], start=False, stop=True)
                    nc.scalar.activation(h_row[s][:, 0:H], ps_h0[:], AF.Copy)
                    transpose_to_hT(h_row[s][:, 0:H], s)
                    if DEBUG and s == 0:
                        nc.sync.dma_start(out=dbg_h0[:], in_=h_row[s][:, 0:H])
                        nc.sync.dma_start(out=dbg_hT[:], in_=hT[s][:])

            # ================= decode steps =================
            logit_sb = [work.tile([P, NSHARD], f32, tag=f"lg{s}", name=f"lg{s}")
                        for s in STREAMS]
            for t in range(NSTEPS):
                for s in STREAMS:
                    # ---- GRU gh half (pre-token; h = h_{t}) ----
                    # psum regions: A rz [0:2F], B i_n [2F:3F], C h_n [3F:4F],
                    # D h_old [4F:5F]
                    ps_g = psg.tile([P, 5 * F], f32, tag=f"g{s}", name=f"g{s}")
                    for k in range(KT):
                        nc.tensor.matmul(
                            ps_g[:, 0:2 * F], lhsT=hT[s][:, k * P:(k + 1) * P],
                            rhs=whh_sb[:, k * 4 * F:k * 4 * F + 2 * F],
                            start=(k == 0), stop=False)
                        nc.tensor.matmul(
                            ps_g[:, 3 * F:5 * F], lhsT=hT[s][:, k * P:(k + 1) * P],
                            rhs=whh_sb[:, k * 4 * F + 2 * F:(k + 1) * 4 * F],
                            start=False, stop=False)
                    if bgate_nz:
                        nc.tensor.matmul(ps_g[:, 3 * F:4 * F], lhsT=ones_col[:],
                                         rhs=bgr_sb[:, 3 * F:4 * F],
                                         start=False, stop=False)

                    # ---- gather x = emb[tok] ----
                    x_sb = sc.tile([P, H], f32, tag=f"x{s}", name=f"x{s}")
                    nc.gpsimd.indirect_dma_start(
                        out=x_sb[:], out_offset=None, in_=emb[:],
                        in_offset=IndirectOffsetOnAxis(ap=tok_sb[s][:, :1], axis=0))
                    ps_tp = pst.tile([P, H], f32, tag=f"tp{s}", name=f"tpx{s}")
                    for j in range(KT):
                        nc.tensor.transpose(ps_tp[:, j * P:(j + 1) * P],
                                            x_sb[:, j * P:(j + 1) * P], ident[:])
                    nc.vector.tensor_copy(xT[s][:], ps_tp[:])

                    # ---- GRU gi half ----
                    for k in range(KT):
                        nc.tensor.matmul(
                            ps_g[:, 0:2 * F], lhsT=xT[s][:, k * P:(k + 1) * P],
                            rhs=wih_sb[:, k * 3 * F:k * 3 * F + 2 * F],
                            start=False, stop=False)
                        nc.tensor.matmul(
                            ps_g[:, 2 * F:3 * F], lhsT=xT[s][:, k * P:(k + 1) * P],
                            rhs=wih_sb[:, k * 3 * F + 2 * F:(k + 1) * 3 * F],
                            start=False, stop=(k == KT - 1 and not bgate_nz))
                    if bgate_nz:
                        nc.tensor.matmul(ps_g[:, 0:2 * F], lhsT=ones_col[:],
                                         rhs=bgr_sb[:, 0:2 * F], start=False, stop=False)
                        nc.tensor.matmul(ps_g[:, 2 * F:3 * F], lhsT=ones_col[:],
                                         rhs=bgr_sb[:, 2 * F:3 * F], start=False, stop=True)

                    # ---- gates elementwise on [128, 64] slices ----
                    r_sb = sc.tile([P, F], f32, tag=f"r{s}", name=f"r{s}")
                    z_sb = sc.tile([P, F], f32, tag=f"z{s}", name=f"z{s}")
                    nc.scalar.activation(r_sb[:], ps_g[:, 0:F], AF.Sigmoid)
                    nc.scalar.activation(z_sb[:], ps_g[:, F:2 * F], AF.Sigmoid)
                    t2_sb = sc.tile([P, F], f32, tag=f"t2{s}", name=f"t2{s}")
                    nc.vector.tensor_mul(t2_sb[:], r_sb[:], ps_g[:, 3 * F:4 * F])
                    nc.vector.tensor_add(t2_sb[:], t2_sb[:], ps_g[:, 2 * F:3 * F])
                    n_sb = sc.tile([P, F], f32, tag=f"n{s}", name=f"n{s}")
                    nc.scalar.activation(n_sb[:], t2_sb[:], AF.Tanh)
                    hold_sb = sc.tile([P, F], f32, tag=f"ho{s}", name=f"ho{s}")
                    nc.scalar.activation(hold_sb[:], ps_g[:, 4 * F:5 * F], AF.Copy)
                    # h' = n + z*(h_old - n)
                    d_sb = sc.tile([P, F], f32, tag=f"d{s}", name=f"d{s}")
                    nc.gpsimd.tensor_sub(d_sb[:], hold_sb[:], n_sb[:])
                    nc.gpsimd.tensor_mul(d_sb[:], d_sb[:], z_sb[:])
                    nc.gpsimd.tensor_add(d_sb[:], d_sb[:], n_sb[:])
                    if DEBUG and t == 0 and s == 0:
                        nc.sync.dma_start(out=dbg_x[:], in_=x_sb[:])
                        nc.sync.dma_start(out=dbg_xT[:], in_=xT[s][:])
                        nc.sync.dma_start(out=dbg_r[:], in_=r_sb[:])
                        nc.sync.dma_start(out=dbg_z[:], in_=z_sb[:])
                        nc.sync.dma_start(out=dbg_n[:], in_=n_sb[:])
                        nc.sync.dma_start(out=dbg_ho[:], in_=hold_sb[:])

                    # ---- share h' slices: AllGather 32KB ----
                    nc.sync.dma_start(
                        out=h_in[t][s][:].rearrange("(p f) -> p f", f=F),
                        in_=d_sb[:])
                    nc.gpsimd.collective_compute(
                        "AllGather", OP.bypass,
                        replica_groups=[list(range(NCORES))],
                        ins=[h_in[t][s][:]], outs=[h_out[t][s][:]])
                    nc.sync.dma_start(
                        out=h_row[s][:, 0:H].rearrange("p (c f) -> p c f", f=F),
                        in_=h_out[t][s][:].rearrange("(c p f) -> p c f", c=NCORES, f=F))
                    transpose_to_hT(h_row[s][:, 0:H], s)

                    # ---- prediction matmul in f32r: logits [128, 4096] ----
                    for n in range(NT):
                        ps_pred = psr.tile([P, 512], f32, tag="pred", name="pred")
                        for k in range(KT):
                            nc.tensor.matmul(
                                ps_pred[:], lhsT=hTr[s][:, k * P:(k + 1) * P],
                                rhs=wpt_r[k][:, n * 512:(n + 1) * 512],
                                start=(k == 0),
                                stop=(k == KT - 1 and not bpred_nz))
                        if bpred_nz:
                            nc.tensor.matmul(
                                ps_pred[:], lhsT=ones_col_r[:],
                                rhs=bpr_sb[:, n * 512:(n + 1) * 512],
                                start=False, stop=True)
                        nc.scalar.activation(logit_sb[s][:, n * 512:(n + 1) * 512],
                                             ps_pred[:], AF.Copy)

                    # ---- local top-8 + exact top-2 rescore ----
                    m8 = sc.tile([P, 8], f32, tag=f"m8{s}", name=f"m8{s}")
                    mi = sc.tile([P, 8], u32, tag=f"mi{s}", name=f"mi{s}")
                    nc.vector.max(out=m8[:], in_=logit_sb[s][:])
                    nc.vector.max_index(out=mi[:], in_max=m8[:], in_values=logit_sb[s][:])
                    w1 = sc.tile([P, H + 1], f32, tag=f"w1{s}", name=f"w1{s}")
                    w2 = sc.tile([P, H + 1], f32, tag=f"w2{s}", name=f"w2{s}")
                    nc.gpsimd.indirect_dma_start(
                        out=w1[:], out_offset=None, in_=wrows[:],
                        in_offset=IndirectOffsetOnAxis(ap=mi[:, 0:1].bitcast(i32), axis=0))
                    nc.gpsimd.indirect_dma_start(
                        out=w2[:], out_offset=None, in_=wrows[:],
                        in_offset=IndirectOffsetOnAxis(ap=mi[:, 1:2].bitcast(i32), axis=0))
                    # exact dots (h_row col H is 1.0, wrows col H is b_pred)
                    nc.vector.tensor_mul(w1[:], w1[:], h_row[s][:])
                    nc.vector.tensor_mul(w2[:], w2[:], h_row[s][:])
                    ev = sc.tile([P, 2], f32, tag=f"ev{s}", name=f"ev{s}")
                    nc.vector.tensor_reduce(ev[:, 0:1], w1[:], AxisX, OP.add)
                    nc.vector.tensor_reduce(ev[:, 1:2], w2[:], AxisX, OP.add)

                    # pick best of the two exact values; tie -> smaller index
                    gi1 = sc.tile([P, 2], i32, tag=f"gi{s}", name=f"gi{s}")
                    nc.vector.tensor_add(gi1[:, 0:1], mi[:, 0:1].bitcast(i32), base_sb[:])
                    nc.vector.tensor_add(gi1[:, 1:2], mi[:, 1:2].bitcast(i32), base_sb[:])
                    key = sc.tile([P, 2], f32, tag=f"key{s}", name=f"key{s}")
                    m_gt = sc.tile([P, 1], u32, tag=f"mgt{s}", name=f"mgt{s}")
                    nc.vector.tensor_tensor(m_gt[:], ev[:, 1:2], ev[:, 0:1], OP.is_gt)
                    m_eq = sc.tile([P, 1], u32, tag=f"meq{s}", name=f"meq{s}")
                    nc.vector.tensor_tensor(m_eq[:], ev[:, 1:2], ev[:, 0:1], OP.is_equal)
                    m_lt = sc.tile([P, 1], u32, tag=f"mlt{s}", name=f"mlt{s}")
                    nc.vector.tensor_tensor(m_lt[:], gi1[:, 1:2], gi1[:, 0:1], OP.is_lt)
                    nc.vector.tensor_mul(m_eq[:], m_eq[:], m_lt[:])
                    nc.vector.tensor_tensor(m_gt[:], m_gt[:], m_eq[:], OP.logical_or)
                    nc.vector.tensor_copy(key[:, 0:1], ev[:, 0:1])
                    nc.vector.copy_predicated(key[:, 0:1], m_gt[:], ev[:, 1:2])
                    nc.vector.tensor_copy(key[:, 1:2].bitcast(i32), gi1[:, 0:1])
                    nc.vector.copy_predicated(key[:, 1:2].bitcast(i32), m_gt[:], gi1[:, 1:2])

                    # ---- cross-core argmax: AllGather + combine ----
                    nc.sync.dma_start(
                        out=k_in[t][s][:].rearrange("(p w) -> p w", w=2),
                        in_=key[:])
                    nc.gpsimd.collective_compute(
                        "AllGather", OP.bypass,
                        replica_groups=[list(range(NCORES))],
                        ins=[k_in[t][s][:]], outs=[k_out[t][s][:]])
                    gv = k_out[t][s][:].rearrange("(c p w) -> p c w", c=NCORES, w=2)
                    vals8 = sc.tile([P, NCORES], f32, tag=f"v8{s}", name=f"v8{s}")
                    idx8 = sc.tile([P, NCORES], i32, tag=f"i8{s}", name=f"i8{s}")
                    nc.sync.dma_start(out=vals8[:], in_=gv[:, :, 0])
                    nc.sync.dma_start(out=idx8[:], in_=gv[:, :, 1].bitcast(i32))
                    gmax = sc.tile([P, 1], f32, tag=f"gm{s}", name=f"gm{s}")
                    nc.vector.tensor_reduce(gmax[:], vals8[:], AxisX, OP.max)
                    mask = sc.tile([P, NCORES], u32, tag=f"mk{s}", name=f"mk{s}")
                    nc.vector.tensor_tensor(mask[:], vals8[:],
                                            gmax[:].to_broadcast([P, NCORES]),
                                            OP.is_ge)
                    cand = sc.tile([P, NCORES], i32, tag=f"cd{s}", name=f"cd{s}")
                    nc.vector.memset(cand[:], 0x7FFFFFFF)
                    nc.vector.copy_predicated(cand[:], mask[:], idx8[:])
                    tok_new = work.tile([P, 1], i32, tag=f"tok{s}", name=f"tok{s}")
                    nc.vector.tensor_reduce(tok_new[:], cand[:], AxisX, OP.min)
                    tok_sb[s] = tok_new
                    if DEBUG and t == 0 and s == 0:
                        nc.sync.dma_start(out=dbg_h[:], in_=h_row[s][:])
                        nc.sync.dma_start(out=dbg_m8[:], in_=m8[:])
                        nc.sync.dma_start(out=dbg_mi[:], in_=mi[:].bitcast(i32))
                        nc.sync.dma_start(out=dbg_ev[:], in_=ev[:])
                        nc.sync.dma_start(out=dbg_key[:], in_=key[:])
                        nc.sync.dma_start(out=dbg_v8[:], in_=vals8[:])
                        nc.sync.dma_start(out=dbg_lg[:], in_=logit_sb[s][:])
                    with nc.allow_non_contiguous_dma(reason="column write, 128x4B"):
                        nc.sync.dma_start(
                            out=preds[s * P:(s + 1) * P, t + 1][:, None],
                            in_=tok_new[:])

    return nc


def _prep_inputs(caption, img, embedding, W_cond, b_cond, w_ih, w_hh, b_ih,
                 b_hh, W_pred, b_pred):
    caption = np.asarray(caption).astype(np.int32)
    img = np.ascontiguousarray(np.asarray(img, dtype=np.float32))
    embedding = np.ascontiguousarray(np.asarray(embedding, dtype=np.float32))
    W_pred = np.asarray(W_pred, dtype=np.float32)
    b_pred = np.asarray(b_pred, np.float32)
    b_ih = np.asarray(b_ih, np.float32)
    b_hh = np.asarray(b_hh, np.float32)
    wihT = np.ascontiguousarray(np.asarray(w_ih, np.float32).T)   # [H, 3H]
    whhT = np.ascontiguousarray(np.asarray(w_hh, np.float32).T)
    common = dict(
        emb=embedding,
        wct=np.ascontiguousarray(np.asarray(W_cond, np.float32).T),
        imgT=np.ascontiguousarray(img.T),
        tok0=np.ascontiguousarray(caption[:, 0]),
        ident_in=np.eye(P, dtype=np.float32),
        bcond_row=np.asarray(b_cond, np.float32).reshape(1, H),
    )
    in_maps = []
    for c in range(NCORES):
        base = c * NSHARD
        hi = min(base + NSHARD, VOCAB)
        n_real = max(0, hi - base)
        wpt_c = np.zeros((H, NSHARD), np.float32)
        wpt_c[:, :n_real] = W_pred[base:hi].T
        wrows_c = np.zeros((NSHARD, H + 1), np.float32)
        wrows_c[:n_real, :H] = W_pred[base:hi]
        wrows_c[:n_real, H] = b_pred[base:hi]
        # permuted gate columns for this core: r_c | z_c | n_c (64 each)
        lo, hi_f = c * F, (c + 1) * F
        wih_r = np.concatenate(
            [wihT[:, lo:hi_f], wihT[:, H + lo:H + hi_f],
             wihT[:, 2 * H + lo:2 * H + hi_f]], axis=1)
        sel = np.zeros((H, F), np.float32)
        sel[lo:hi_f, :] = np.eye(F, dtype=np.float32)
        whh_r = np.concatenate(
            [whhT[:, lo:hi_f], whhT[:, H + lo:H + hi_f],
             whhT[:, 2 * H + lo:2 * H + hi_f], sel], axis=1)
        bg = b_ih + b_hh
        bgate_row = np.concatenate(
            [bg[lo:hi_f], bg[H + lo:H + hi_f],
             b_ih[2 * H + lo:2 * H + hi_f], b_hh[2 * H + lo:2 * H + hi_f]])
        m = dict(common)
        m["wpt"] = np.ascontiguousarray(wpt_c)
        m["wrows"] = np.ascontiguousarray(wrows_c)
        m["wih_r"] = np.ascontiguousarray(wih_r)
        m["whh_r"] = np.ascontiguousarray(whh_r)
        m["base_t"] = np.full((P, 1), base, np.int32)
        m["bgate_row"] = np.ascontiguousarray(bgate_row.reshape(1, 4 * F))
        bp = np.zeros((1, NSHARD), np.float32)
        bp[0, :n_real] = b_pred[base:hi]
        m["bpred_row"] = bp
        in_maps.append(m)
    return in_maps


_CACHED = {}


def kernel(**inputs) -> np.ndarray:
    from concourse.bass_utils import run_bass_kernel_spmd

    in_maps = _prep_inputs(**inputs)
    bcond_nz = bool(np.any(np.asarray(inputs["b_cond"])))
    bgate_nz = bool(np.any(np.asarray(inputs["b_ih"]))
                    or np.any(np.asarray(inputs["b_hh"])))
    bpred_nz = bool(np.any(np.asarray(inputs["b_pred"])))
    key = (bcond_nz, bgate_nz, bpred_nz)
    if key not in _CACHED:
        nc = _build(*key)
        nc.finalize()
        _CACHED[key] = nc
    flags = ("bcond_row", "bgate_row", "bpred_row")
    drop = [f for f, nz in zip(flags, key) if not nz]
    for m in in_maps:
        for f in drop:
            m.pop(f, None)
    res = run_bass_kernel_spmd(_CACHED[key], in_maps, list(range(NCORES)))
    return np.ascontiguousarray(res.results[0]["preds"].astype(np.int32))


if __name__ == "__main__":
    d = np.load("inputs.npz")
    inputs = {k: d[k] for k in d.files}
    out = kernel(**inputs)
    exp = np.load("expected.npy")
    print("match:", np.array_equal(out, exp),
          " mismatches:", int((out != exp).sum()), "/", out.size)


# revision 7
# speedup vs baseline: 1.4761x; 1.0108x over previous
"""ConditionalLM decode kernel for 8 Trainium2 NeuronCores.

Strategy (v4):
  - Vocab-shard W_pred across 8 cores (4096 cols each, zero-padded); the
    prediction matmul runs in float32r (1 cyc/row vs 4 for fp32); per-block
    top-8 scans ride under the matmul, and the top-2 local candidates are
    re-scored with an exact fp32 dot product so fp32r rounding (~3.5e-7
    logit noise) cannot flip the argmax (top-2 margins go down to 5e-8).
  - GRU sharded 8 ways by gate feature columns (64 h-features per core,
    permuted weight columns r_c|z_c|n_c per core), row-major exact fp32;
    h' slices exchanged with a 32KB AllGather per stream-step.  h_old is
    selected with a one-hot matrix appended to the whh upload (SPMD
    programs cannot use per-core addresses).  PSUM zero regions are 2KB:
    exactly one start=True per bank per step.
  - Batch split into 2 streams of 128 rows; emission is interleaved per
    sub-phase so engine FIFOs never head-of-line block the other stream.
  - Cross-core argmax: (exact val, global idx) pairs, AllGather + local
    combine; ties resolve to the smallest vocab index, matching jnp.argmax.
"""
import numpy as np

VOCAB = 32002
H = 512
COND = 1024
MAXLEN = 15
B = 256
NCORES = 8
NSHARD = 4096          # per-core vocab shard (8*4096 = 32768 >= 32002)
NSTEPS = MAXLEN - 1    # 14 decode steps
P = 128
F = 64                 # h-features per core (512/8)
STREAMS = (0, 1)
KT = 4                 # hidden k-tiles (512/128)
KC = 8                 # cond k-tiles (1024/128)
NT = NSHARD // 512     # 8 pred n-tiles


def _build(bcond_nz=False, bgate_nz=False, bpred_nz=False):
    import concourse.bacc as bacc
    import concourse.mybir as mybir
    from concourse.tile import TileContext
    from concourse.bass import IndirectOffsetOnAxis

    f32 = mybir.dt.float32
    f32r = mybir.dt.float32r
    i32 = mybir.dt.int32
    u32 = mybir.dt.uint32
    AF = mybir.ActivationFunctionType
    OP = mybir.AluOpType
    AxisX = mybir.AxisListType.X

    nc = bacc.Bacc("TRN2", target_bir_lowering=False, debug=True, num_devices=NCORES)

    # ---------------- I/O ----------------
    emb = nc.declare_dram_parameter("emb", [VOCAB, H], f32, isOutput=False)
    wpt = nc.declare_dram_parameter("wpt", [H, NSHARD], f32, isOutput=False)
    wrows = nc.declare_dram_parameter("wrows", [NSHARD, H + 1], f32, isOutput=False)
    wih_r = nc.declare_dram_parameter("wih_r", [H, 3 * F], f32, isOutput=False)
    whh_r = nc.declare_dram_parameter("whh_r", [H, 3 * F + F], f32, isOutput=False)
    wct = nc.declare_dram_parameter("wct", [COND, H], f32, isOutput=False)
    imgT_d = nc.declare_dram_parameter("imgT", [COND, B], f32, isOutput=False)
    tok0 = nc.declare_dram_parameter("tok0", [B], i32, isOutput=False)
    base_t = nc.declare_dram_parameter("base_t", [P, 1], i32, isOutput=False)
    ident_in = nc.declare_dram_parameter("ident_in", [P, P], f32, isOutput=False)
    if bcond_nz:
        bcond_row = nc.declare_dram_parameter("bcond_row", [1, H], f32, isOutput=False)
    if bgate_nz:
        bgate_row = nc.declare_dram_parameter("bgate_row", [1, 4 * F], f32, isOutput=False)
    if bpred_nz:
        bpred_row = nc.declare_dram_parameter("bpred_row", [1, NSHARD], f32, isOutput=False)
    preds = nc.declare_dram_parameter("preds", [B, MAXLEN], i32, isOutput=True)

    # internal DRAM for collectives (one pair per stream-step, static)
    k_in = [[nc.dram_tensor(f"k_in_{t}_{s}", [P * 2], f32) for s in STREAMS]
            for t in range(NSTEPS)]
    k_out = [[nc.dram_tensor(f"k_out_{t}_{s}", [NCORES * P * 2], f32,
                             addr_space="Shared") for s in STREAMS]
             for t in range(NSTEPS)]
    h_in = [[nc.dram_tensor(f"h_in_{t}_{s}", [P * F], f32) for s in STREAMS]
            for t in range(NSTEPS)]
    h_out = [[nc.dram_tensor(f"h_out_{t}_{s}", [NCORES * P * F], f32,
                             addr_space="Shared") for s in STREAMS]
             for t in range(NSTEPS)]

    with TileContext(nc) as tc:
        with (
            tc.tile_pool(name="wts", bufs=1) as wts,
            tc.tile_pool(name="work", bufs=1) as work,
            tc.tile_pool(name="sc", bufs=1) as sc,
            tc.tile_pool(name="psg", bufs=1, space="PSUM") as psg,
            tc.tile_pool(name="pst", bufs=1, space="PSUM") as pst,
            tc.tile_pool(name="psr", bufs=5, space="PSUM") as psr,
        ):
            # ================= resident weights =================
            base_sb = wts.tile([P, 1], i32, tag="base", name="base")
            nc.sync.dma_start(out=base_sb[:], in_=base_t[:])
            ident = wts.tile([P, P], f32, tag="ident", name="ident")
            nc.sync.dma_start(out=ident[:], in_=ident_in[:])

            wih_sb = wts.tile([P, KT * 3 * F], f32, tag="wih", name="wih")
            whh_sb = wts.tile([P, KT * 4 * F], f32, tag="whh", name="whh")
            for k in range(KT):
                nc.sync.dma_start(out=wih_sb[:, k * 3 * F:(k + 1) * 3 * F],
                                  in_=wih_r[k * P:(k + 1) * P, :])
                nc.sync.dma_start(out=whh_sb[:, k * 4 * F:(k + 1) * 4 * F],
                                  in_=whh_r[k * P:(k + 1) * P, :])

            # prediction weights: stage fp32, cast to f32r
            wpt_r = [wts.tile([P, NSHARD], f32r, tag=f"wptr{k}", name=f"wptr{k}")
                     for k in range(KT)]
            with tc.tile_pool(name="stage", bufs=2) as stage:
                for k in range(KT):
                    st = stage.tile([P, NSHARD], f32, tag="st", name="st")
                    nc.sync.dma_start(out=st[:], in_=wpt[k * P:(k + 1) * P, :])
                    nc.vector.tensor_copy(wpt_r[k][:], st[:])

            if bcond_nz or bgate_nz or bpred_nz:
                ones_col = wts.tile([1, P], f32, tag="ones", name="ones")
                nc.vector.memset(ones_col[:], 1.0)
            if bcond_nz:
                bcr_sb = wts.tile([1, H], f32, tag="bcr", name="bcr")
                nc.sync.dma_start(out=bcr_sb[:], in_=bcond_row[:])
            if bgate_nz:
                bgr_sb = wts.tile([1, 4 * F], f32, tag="bgr", name="bgr")
                nc.sync.dma_start(out=bgr_sb[:], in_=bgate_row[:])
            if bpred_nz:
                ones_col_r = wts.tile([1, P], f32r, tag="onesr", name="onesr")
                nc.vector.memset(ones_col_r[:], 1.0)
                bpr_sb = wts.tile([1, NSHARD], f32r, tag="bpr", name="bpr")
                st2 = wts.tile([1, NSHARD], f32, tag="bprs", name="bprs")
                nc.sync.dma_start(out=st2[:], in_=bpred_row[:])
                nc.vector.tensor_copy(bpr_sb[:], st2[:])

            tok_sb = [work.tile([P, 1], i32, tag=f"tok{s}", name=f"tok{s}")
                      for s in STREAMS]
            tokh = [work.tile([P, MAXLEN], i32, tag=f"tokh{s}", name=f"tokh{s}")
                    for s in STREAMS]
            for s in STREAMS:
                nc.sync.dma_start(out=tok_sb[s][:], in_=tok0[s * P:(s + 1) * P][:, None])
                nc.vector.tensor_copy(tokh[s][:, 0:1], tok_sb[s][:])

            # persistent state (h_row col H holds 1.0 for the bias-augmented
            # rescore dot)
            h_row = [work.tile([P, H + 1], f32, tag=f"hrow{s}", name=f"hrow{s}")
                     for s in STREAMS]
            hT = [work.tile([P, H], f32, tag=f"hT{s}", name=f"hT{s}")
                  for s in STREAMS]
            hTr = [work.tile([P, H], f32r, tag=f"hTr{s}", name=f"hTr{s}")
                   for s in STREAMS]
            xT = [work.tile([P, H], f32, tag=f"xT{s}", name=f"xT{s}")
                  for s in STREAMS]
            for s in STREAMS:
                nc.vector.memset(h_row[s][:, H:H + 1], 1.0)

            def transpose_to_hT(src_row, s):
                ps_tp = pst.tile([P, H], f32, tag="tp", name=f"tp{s}")
                for j in range(KT):
                    nc.tensor.transpose(ps_tp[:, j * P:(j + 1) * P],
                                        src_row[:, j * P:(j + 1) * P], ident[:])
                nc.scalar.activation(hT[s][:], ps_tp[:], AF.Copy)
                nc.vector.tensor_copy(hTr[s][:], ps_tp[:])

            # ================= h0 = img @ W_cond.T (row-major) =================
            with tc.tile_pool(name="setup", bufs=1) as setup:
                wct_sb = [setup.tile([P, H], f32, tag=f"wct{k}", name=f"wct{k}")
                          for k in range(KC)]
                imgT_sb = [setup.tile([P, B], f32, tag=f"img{k}", name=f"img{k}")
                           for k in range(KC)]
                for k in range(KC):
                    nc.sync.dma_start(out=wct_sb[k][:], in_=wct[k * P:(k + 1) * P, :])
                    nc.sync.dma_start(out=imgT_sb[k][:], in_=imgT_d[k * P:(k + 1) * P, :])
                for s in STREAMS:
                    ps_h0 = psr.tile([P, H], f32, tag="pred", name="ps_h0")
                    for k in range(KC):
                        nc.tensor.matmul(
                            ps_h0[:], lhsT=imgT_sb[k][:, s * P:(s + 1) * P],
                            rhs=wct_sb[k][:], start=(k == 0),
                            stop=(k == KC - 1 and not bcond_nz))
                    if bcond_nz:
                        nc.tensor.matmul(ps_h0[:], lhsT=ones_col[:],
                                         rhs=bcr_sb[:], start=False, stop=True)
                    nc.scalar.activation(h_row[s][:, 0:H], ps_h0[:], AF.Copy)
                    transpose_to_hT(h_row[s][:, 0:H], s)

            # ================= decode steps =================
            ps_g = [None, None]

            def emit_gh(t, s):
                ps_g[s] = psg.tile([P, 5 * F], f32, tag=f"g{s}", name=f"g{s}")
                pg = ps_g[s]
                for k in range(KT):
                    nc.tensor.matmul(
                        pg[:, 0:2 * F], lhsT=hT[s][:, k * P:(k + 1) * P],
                        rhs=whh_sb[:, k * 4 * F:k * 4 * F + 2 * F],
                        start=(k == 0), stop=False)
                    nc.tensor.matmul(
                        pg[:, 3 * F:5 * F], lhsT=hT[s][:, k * P:(k + 1) * P],
                        rhs=whh_sb[:, k * 4 * F + 2 * F:(k + 1) * 4 * F],
                        start=False, stop=False)

            def emit_gather_x(t, s):
                x_sb = sc.tile([P, H], f32, tag=f"x{s}", name=f"x{s}")
                nc.gpsimd.indirect_dma_start(
                    out=x_sb[:], out_offset=None, in_=emb[:],
                    in_offset=IndirectOffsetOnAxis(ap=tok_sb[s][:, :1], axis=0))
                ps_tp = pst.tile([P, H], f32, tag="tp", name=f"tpx{s}")
                for j in range(KT):
                    nc.tensor.transpose(ps_tp[:, j * P:(j + 1) * P],
                                        x_sb[:, j * P:(j + 1) * P], ident[:])
                nc.vector.tensor_copy(xT[s][:], ps_tp[:])

            def emit_gi(t, s):
                pg = ps_g[s]
                for k in range(KT):
                    nc.tensor.matmul(
                        pg[:, 0:2 * F], lhsT=xT[s][:, k * P:(k + 1) * P],
                        rhs=wih_sb[:, k * 3 * F:k * 3 * F + 2 * F],
                        start=False, stop=False)
                    nc.tensor.matmul(
                        pg[:, 2 * F:3 * F], lhsT=xT[s][:, k * P:(k + 1) * P],
                        rhs=wih_sb[:, k * 3 * F + 2 * F:(k + 1) * 3 * F],
                        start=False, stop=(k == KT - 1 and not bgate_nz))
                if bgate_nz:
                    nc.tensor.matmul(pg[:, 0:2 * F], lhsT=ones_col[:],
                                     rhs=bgr_sb[:, 0:2 * F], start=False, stop=False)
                    nc.tensor.matmul(pg[:, 2 * F:3 * F], lhsT=ones_col[:],
                                     rhs=bgr_sb[:, 2 * F:3 * F], start=False, stop=False)
                    nc.tensor.matmul(pg[:, 3 * F:4 * F], lhsT=ones_col[:],
                                     rhs=bgr_sb[:, 3 * F:4 * F], start=False, stop=True)

            def emit_gates(t, s):
                pg = ps_g[s]
                r_sb = sc.tile([P, F], f32, tag=f"r{s}", name=f"r{s}")
                z_sb = sc.tile([P, F], f32, tag=f"z{s}", name=f"z{s}")
                nc.scalar.activation(r_sb[:], pg[:, 0:F], AF.Sigmoid)
                nc.scalar.activation(z_sb[:], pg[:, F:2 * F], AF.Sigmoid)
                t2_sb = sc.tile([P, F], f32, tag=f"t2{s}", name=f"t2{s}")
                nc.vector.tensor_mul(t2_sb[:], r_sb[:], pg[:, 3 * F:4 * F])
                nc.vector.tensor_add(t2_sb[:], t2_sb[:], pg[:, 2 * F:3 * F])
                n_sb = sc.tile([P, F], f32, tag=f"n{s}", name=f"n{s}")
                nc.scalar.activation(n_sb[:], t2_sb[:], AF.Tanh)
                # h' = n + z*(h_old - n)
                d_sb = sc.tile([P, F], f32, tag=f"d{s}", name=f"d{s}")
                nc.vector.tensor_sub(d_sb[:], pg[:, 4 * F:5 * F], n_sb[:])
                nc.vector.tensor_mul(d_sb[:], d_sb[:], z_sb[:])
                nc.vector.tensor_add(d_sb[:], d_sb[:], n_sb[:])
                return d_sb

            def emit_hshare(t, s, d_sb):
                nc.sync.dma_start(
                    out=h_in[t][s][:].rearrange("(p f) -> p f", f=F),
                    in_=d_sb[:])
                nc.gpsimd.collective_compute(
                    "AllGather", OP.bypass,
                    replica_groups=[list(range(NCORES))],
                    ins=[h_in[t][s][:]], outs=[h_out[t][s][:]])

            def emit_hback(t, s):
                nc.sync.dma_start(
                    out=h_row[s][:, 0:H].rearrange("p (c f) -> p c f", f=F),
                    in_=h_out[t][s][:].rearrange("(c p f) -> p c f", c=NCORES, f=F))
                transpose_to_hT(h_row[s][:, 0:H], s)

            def emit_pred(t, s):
                """fp32r pred with per-block top-8 scans riding under it."""
                bt2 = sc.tile([P, 2 * NT], f32, tag=f"bt2{s}", name=f"bt2{s}")
                bi2 = sc.tile([P, 2 * NT], i32, tag=f"bi2{s}", name=f"bi2{s}")
                for n in range(NT):
                    ps_pred = psr.tile([P, 512], f32, tag="pred", name="pred")
                    for k in range(KT):
                        nc.tensor.matmul(
                            ps_pred[:], lhsT=hTr[s][:, k * P:(k + 1) * P],
                            rhs=wpt_r[k][:, n * 512:(n + 1) * 512],
                            start=(k == 0),
                            stop=(k == KT - 1 and not bpred_nz))
                    if bpred_nz:
                        nc.tensor.matmul(
                            ps_pred[:], lhsT=ones_col_r[:],
                            rhs=bpr_sb[:, n * 512:(n + 1) * 512],
                            start=False, stop=True)
                    bm = sc.tile([P, 8], f32, tag=f"bm{s}_{n % 2}", name=f"bm{s}_{n}")
                    bi = sc.tile([P, 8], u32, tag=f"bi{s}_{n % 2}", name=f"bi{s}_{n}")
                    nc.vector.max(out=bm[:], in_=ps_pred[:])
                    nc.vector.max_index(out=bi[:], in_max=bm[:], in_values=ps_pred[:])
                    nc.vector.tensor_copy(bt2[:, 2 * n:2 * n + 2], bm[:, 0:2])
                    nc.vector.tensor_scalar_add(bi2[:, 2 * n:2 * n + 2],
                                                bi[:, 0:2].bitcast(i32), n * 512)
                return bt2, bi2

            def emit_argmax_rescore(t, s, bt2, bi2):
                # local top-2 across the 8 blocks
                g8 = sc.tile([P, 8], f32, tag=f"g8{s}", name=f"g8{s}")
                nc.vector.max(out=g8[:], in_=bt2[:])
                li = sc.tile([P, 2], i32, tag=f"li{s}", name=f"li{s}")
                for j in range(2):
                    msk = sc.tile([P, 2 * NT], u32, tag=f"msk{s}", name=f"msk{s}")
                    nc.vector.tensor_tensor(msk[:], bt2[:],
                                            g8[:, j:j + 1].to_broadcast([P, 2 * NT]),
                                            OP.is_ge)
                    cnd = sc.tile([P, 2 * NT], i32, tag=f"cnd{s}", name=f"cnd{s}")
                    nc.vector.memset(cnd[:], 0x7FFFFFFF)
                    nc.vector.copy_predicated(cnd[:], msk[:], bi2[:])
                    nc.vector.tensor_reduce(li[:, j:j + 1], cnd[:], AxisX, OP.min)
                # exact rescore of both candidates
                w1 = sc.tile([P, H + 1], f32, tag=f"w1{s}", name=f"w1{s}")
                w2 = sc.tile([P, H + 1], f32, tag=f"w2{s}", name=f"w2{s}")
                nc.gpsimd.indirect_dma_start(
                    out=w1[:], out_offset=None, in_=wrows[:],
                    in_offset=IndirectOffsetOnAxis(ap=li[:, 0:1], axis=0))
                nc.gpsimd.indirect_dma_start(
                    out=w2[:], out_offset=None, in_=wrows[:],
                    in_offset=IndirectOffsetOnAxis(ap=li[:, 1:2], axis=0))
                nc.vector.tensor_mul(w1[:], w1[:], h_row[s][:])
                nc.vector.tensor_mul(w2[:], w2[:], h_row[s][:])
                ev = sc.tile([P, 2], f32, tag=f"ev{s}", name=f"ev{s}")
                nc.vector.tensor_reduce(ev[:, 0:1], w1[:], AxisX, OP.add)
                nc.vector.tensor_reduce(ev[:, 1:2], w2[:], AxisX, OP.add)
                # key = (max exact val, its global idx; tie -> min idx)
                gi2 = sc.tile([P, 2], i32, tag=f"gi2{s}", name=f"gi2{s}")
                nc.vector.tensor_add(gi2[:, 0:1], li[:, 0:1], base_sb[:])
                nc.vector.tensor_add(gi2[:, 1:2], li[:, 1:2], base_sb[:])
                key = sc.tile([P, 2], f32, tag=f"key{s}", name=f"key{s}")
                nc.vector.tensor_reduce(key[:, 0:1], ev[:], AxisX, OP.max)
                mk2 = sc.tile([P, 2], u32, tag=f"mk2{s}", name=f"mk2{s}")
                nc.vector.tensor_tensor(mk2[:], ev[:],
                                        key[:, 0:1].to_broadcast([P, 2]), OP.is_ge)
                cn2 = sc.tile([P, 2], i32, tag=f"cn2{s}", name=f"cn2{s}")
                nc.vector.memset(cn2[:], 0x7FFFFFFF)
                nc.vector.copy_predicated(cn2[:], mk2[:], gi2[:])
                nc.vector.tensor_reduce(key[:, 1:2].bitcast(i32), cn2[:], AxisX, OP.min)
                nc.sync.dma_start(
                    out=k_in[t][s][:].rearrange("(p w) -> p w", w=2),
                    in_=key[:])
                nc.gpsimd.collective_compute(
                    "AllGather", OP.bypass,
                    replica_groups=[list(range(NCORES))],
                    ins=[k_in[t][s][:]], outs=[k_out[t][s][:]])

            def emit_combine(t, s):
                gv = k_out[t][s][:].rearrange("(c p w) -> p c w", c=NCORES, w=2)
                vals8 = sc.tile([P, NCORES], f32, tag=f"v8{s}", name=f"v8{s}")
                idx8 = sc.tile([P, NCORES], i32, tag=f"i8{s}", name=f"i8{s}")
                nc.sync.dma_start(out=vals8[:], in_=gv[:, :, 0])
                nc.sync.dma_start(out=idx8[:], in_=gv[:, :, 1].bitcast(i32))
                gmax = sc.tile([P, 1], f32, tag=f"gm{s}", name=f"gm{s}")
                nc.vector.tensor_reduce(gmax[:], vals8[:], AxisX, OP.max)
                mask = sc.tile([P, NCORES], u32, tag=f"mk{s}", name=f"mk{s}")
                nc.vector.tensor_tensor(mask[:], vals8[:],
                                        gmax[:].to_broadcast([P, NCORES]), OP.is_ge)
                cand = sc.tile([P, NCORES], i32, tag=f"cd{s}", name=f"cd{s}")
                nc.vector.memset(cand[:], 0x7FFFFFFF)
                nc.vector.copy_predicated(cand[:], mask[:], idx8[:])
                tok_new = work.tile([P, 1], i32, tag=f"tok{s}", name=f"tok{s}")
                nc.vector.tensor_reduce(tok_new[:], cand[:], AxisX, OP.min)
                tok_sb[s] = tok_new
                nc.vector.tensor_copy(tokh[s][:, t + 1:t + 2], tok_new[:])

            d_cur = [None, None]
            bt_cur = [None, None]
            for t in range(NSTEPS):
                for s in STREAMS:
                    emit_gh(t, s)
                for s in STREAMS:
                    emit_gather_x(t, s)
                for s in STREAMS:
                    emit_gi(t, s)
                for s in STREAMS:
                    d_cur[s] = emit_gates(t, s)
                for s in STREAMS:
                    emit_hshare(t, s, d_cur[s])
                for s in STREAMS:
                    emit_hback(t, s)
                for s in STREAMS:
                    bt_cur[s] = emit_pred(t, s)
                for s in STREAMS:
                    emit_argmax_rescore(t, s, *bt_cur[s])
                for s in STREAMS:
                    emit_combine(t, s)

            # final: write predictions once per stream
            for s in STREAMS:
                nc.sync.dma_start(out=preds[s * P:(s + 1) * P, :], in_=tokh[s][:])

    return nc


def _prep_inputs(caption, img, embedding, W_cond, b_cond, w_ih, w_hh, b_ih,
                 b_hh, W_pred, b_pred):
    caption = np.asarray(caption).astype(np.int32)
    img = np.ascontiguousarray(np.asarray(img, dtype=np.float32))
    embedding = np.ascontiguousarray(np.asarray(embedding, dtype=np.float32))
    W_pred = np.asarray(W_pred, dtype=np.float32)
    b_pred = np.asarray(b_pred, np.float32)
    b_ih = np.asarray(b_ih, np.float32)
    b_hh = np.asarray(b_hh, np.float32)
    wihT = np.ascontiguousarray(np.asarray(w_ih, np.float32).T)   # [H, 3H]
    whhT = np.ascontiguousarray(np.asarray(w_hh, np.float32).T)
    common = dict(
        emb=embedding,
        wct=np.ascontiguousarray(np.asarray(W_cond, np.float32).T),
        imgT=np.ascontiguousarray(img.T),
        tok0=np.ascontiguousarray(caption[:, 0]),
        ident_in=np.eye(P, dtype=np.float32),
        bcond_row=np.asarray(b_cond, np.float32).reshape(1, H),
    )
    in_maps = []
    for c in range(NCORES):
        base = c * NSHARD
        hi = min(base + NSHARD, VOCAB)
        n_real = max(0, hi - base)
        wpt_c = np.zeros((H, NSHARD), np.float32)
        wpt_c[:, :n_real] = W_pred[base:hi].T
        wrows_c = np.zeros((NSHARD, H + 1), np.float32)
        wrows_c[:n_real, :H] = W_pred[base:hi]
        wrows_c[:n_real, H] = b_pred[base:hi]
        lo, hi_f = c * F, (c + 1) * F
        wih_rc = np.concatenate(
            [wihT[:, lo:hi_f], wihT[:, H + lo:H + hi_f],
             wihT[:, 2 * H + lo:2 * H + hi_f]], axis=1)
        sel = np.zeros((H, F), np.float32)
        sel[lo:hi_f, :] = np.eye(F, dtype=np.float32)
        whh_rc = np.concatenate(
            [whhT[:, lo:hi_f], whhT[:, H + lo:H + hi_f],
             whhT[:, 2 * H + lo:2 * H + hi_f], sel], axis=1)
        bg = b_ih + b_hh
        bgate_row = np.concatenate(
            [bg[lo:hi_f], bg[H + lo:H + hi_f],
             b_ih[2 * H + lo:2 * H + hi_f], b_hh[2 * H + lo:2 * H + hi_f]])
        m = dict(common)
        m["wpt"] = np.ascontiguousarray(wpt_c)
        m["wrows"] = np.ascontiguousarray(wrows_c)
        m["wih_r"] = np.ascontiguousarray(wih_rc)
        m["whh_r"] = np.ascontiguousarray(whh_rc)
        m["base_t"] = np.full((P, 1), base, np.int32)
        m["bgate_row"] = np.ascontiguousarray(bgate_row.reshape(1, 4 * F))
        bp = np.zeros((1, NSHARD), np.float32)
        bp[0, :n_real] = b_pred[base:hi]
        m["bpred_row"] = bp
        in_maps.append(m)
    return in_maps


_CACHED = {}


def kernel(**inputs) -> np.ndarray:
    from concourse.bass_utils import run_bass_kernel_spmd

    in_maps = _prep_inputs(**inputs)
    bcond_nz = bool(np.any(np.asarray(inputs["b_cond"])))
    bgate_nz = bool(np.any(np.asarray(inputs["b_ih"]))
                    or np.any(np.asarray(inputs["b_hh"])))
    bpred_nz = bool(np.any(np.asarray(inputs["b_pred"])))
    key = (bcond_nz, bgate_nz, bpred_nz)
    if key not in _CACHED:
        nc = _build(*key)
        nc.finalize()
        _CACHED[key] = nc
    flags = ("bcond_row", "bgate_row", "bpred_row")
    drop = [f for f, nz in zip(flags, key) if not nz]
    for m in in_maps:
        for f in drop:
            m.pop(f, None)
    res = run_bass_kernel_spmd(_CACHED[key], in_maps, list(range(NCORES)))
    return np.ascontiguousarray(res.results[0]["preds"].astype(np.int32))


if __name__ == "__main__":
    d = np.load("inputs.npz")
    inputs = {k: d[k] for k in d.files}
    out = kernel(**inputs)
    exp = np.load("expected.npy")
    print("match:", np.array_equal(out, exp),
          " mismatches:", int((out != exp).sum()), "/", out.size)
